# revision 12
# baseline (speedup 1.0000x reference)
"""Banded local-attention kernel for 8 Trainium2 NeuronCores.

Sharding: batch*seq split into 8 chunks of 1024 rows (2 batches x 4 chunks);
each core gets its rows plus a +-128-row halo of x (transposed, fp16) and
computes its full output slice locally -- no cross-core reduction.

Per-core algorithm (all matmul operands fp16, accumulation f32 in PSUM):
  QT = (Wq x)^T + bq       [512,1024]  (layout: emb_out on partitions)
  KT = (Wk x_halo)^T + bk  [512,1280]
  V  = x_halo Wv^T + outer(hv, bv)  [1280, 512+aug]   (hv = halo-validity 0/1)
  per (q-block of 128 rows, head):
    energyT[j,i] = K^T q   for 3 j-chunks of 128     (PSUM [128,384])
    expT = exp(scale * energyT)  (ACT, fp16 out)
    chunk0 *= lower-tri mask, chunk2 *= upper-tri mask  (band |i-j|<=128)
    out_aug[d,i] = sum_j v_aug[j,d] expT[j,i]  -- aug row = hv gives denominator
    outT[d,i] = out_aug[:64] * (1/out_aug[64])  (partition-broadcast recip)
  final = outT^T Wo^T + bo -> [1024, 512] f32
"""

import numpy as np

import concourse.bass as bass
from concourse import bacc
import concourse.mybir as mybir
from concourse.tile import TileContext
from concourse.bass_utils import run_bass_kernel_spmd
from concourse.masks import make_upper_triangular, make_lower_triangular

EMB = 512
HEADS = 8
WINDOW = 128
BATCH = 2
SEQ = 4096
NCORES = 8
ROWS = 1024            # query rows per core
HALO = ROWS + 2 * WINDOW  # 1280 k/v rows per core
NBLK = ROWS // 128     # 8 q-blocks per core
SCALE = float(1.0 / np.sqrt(EMB))
F16 = mybir.dt.float16
F32 = mybir.dt.float32


def build_bass(mode="full"):
    nc = bacc.Bacc()
    xT = nc.dram_tensor("xT", [EMB, HALO], F16, kind="ExternalInput")
    hvrow = nc.dram_tensor("hvrow", [1, HALO], F16, kind="ExternalInput")
    hvT8 = nc.dram_tensor("hvT8", [128, 80], F16, kind="ExternalInput")
    wqT = nc.dram_tensor("wqT", [EMB, EMB], F16, kind="ExternalInput")
    wkT = nc.dram_tensor("wkT", [EMB, EMB], F16, kind="ExternalInput")
    wvT = nc.dram_tensor("wvT", [EMB, EMB], F16, kind="ExternalInput")
    woT = nc.dram_tensor("woT", [EMB, EMB], F16, kind="ExternalInput")
    bqc = nc.dram_tensor("bqc", [128, 4], F32, kind="ExternalInput")
    bkc = nc.dram_tensor("bkc", [128, 4], F32, kind="ExternalInput")
    bvr = nc.dram_tensor("bvr", [1, EMB], F16, kind="ExternalInput")
    bor = nc.dram_tensor("bor", [1, EMB], F16, kind="ExternalInput")
    out = nc.dram_tensor("out", [ROWS, EMB], F32, kind="ExternalOutput")

    with TileContext(nc) as tc:
        with (
            tc.tile_pool(name="const", bufs=1) as cpool,
            tc.tile_pool(name="big", bufs=1) as bpool,
            tc.tile_pool(name="work", bufs=4) as wpool,
            tc.tile_pool(name="fin", bufs=3) as fpool,
            tc.tile_pool(name="pproj", bufs=2, space="PSUM") as pproj,
            tc.tile_pool(name="pe", bufs=2, space="PSUM") as pepool,
            tc.tile_pool(name="po", bufs=2, space="PSUM") as popool,
        ):
            # ---------------- constants / inputs to SBUF ----------------
            wq_sb, wk_sb, wv_sb, wo_sb = (
                [cpool.tile([128, EMB], F16, tag=f"w{nm}{k}", name=f"w{nm}{k}") for k in range(4)]
                for nm in "qkvo"
            )
            for k in range(4):
                nc.sync.dma_start(wq_sb[k][:], wqT[128 * k:128 * k + 128, :])
                nc.sync.dma_start(wk_sb[k][:], wkT[128 * k:128 * k + 128, :])
                nc.sync.dma_start(wv_sb[k][:], wvT[128 * k:128 * k + 128, :])
                nc.sync.dma_start(wo_sb[k][:], woT[128 * k:128 * k + 128, :])
            xT_sb = [bpool.tile([128, HALO], F16, tag=f"xT{k}", name=f"xT{k}") for k in range(4)]
            for k in range(4):
                nc.sync.dma_start(xT_sb[k][:], xT[128 * k:128 * k + 128, :])
            hv_sb = cpool.tile([1, HALO], F16, tag="hv")
            nc.sync.dma_start(hv_sb[:], hvrow[:])
            hvT8_sb = cpool.tile([128, 80], F16, tag="hvT8")
            nc.sync.dma_start(hvT8_sb[:], hvT8[:])
            bq_sb = cpool.tile([128, 4], F32, tag="bq")
            bk_sb = cpool.tile([128, 4], F32, tag="bk")
            nc.sync.dma_start(bq_sb[:], bqc[:])
            nc.sync.dma_start(bk_sb[:], bkc[:])
            bv_sb = cpool.tile([1, EMB], F16, tag="bv")
            bo_sb = cpool.tile([1, EMB], F16, tag="bo")
            nc.sync.dma_start(bv_sb[:], bvr[:])
            nc.sync.dma_start(bo_sb[:], bor[:])
            ones_row = cpool.tile([1, 128], F16, tag="ones")
            nc.gpsimd.memset(ones_row[:], 1.0)
            negs_row = cpool.tile([1, 64], F16, tag="negs")
            nc.gpsimd.memset(negs_row[:], -1.0)
            # band masks in [j, i] layout: chunk0 valid j>=i, chunk2 valid j<=i
            m0 = cpool.tile([128, 128], F16, tag="m0")
            m2 = cpool.tile([128, 128], F16, tag="m2")
            make_lower_triangular(nc, m0[:], val=1.0, diag=True)
            make_upper_triangular(nc, m2[:], val=1.0, diag=True)

            # ---------------- Q/K projections (transposed layout) --------
            qT_sb = [bpool.tile([128, ROWS], F16, tag=f"qT{m}", name=f"qT{m}") for m in range(4)]
            kT_sb = [bpool.tile([128, HALO], F16, tag=f"kT{m}", name=f"kT{m}") for m in range(4)]
            for m in range(4):
                for off in (0, 512):  # own rows: halo cols 128..1152
                    ps = pproj.tile([128, 512], F32, tag="ps")
                    for kc in range(4):
                        nc.tensor.matmul(
                            ps[:], wq_sb[kc][:, 128 * m:128 * m + 128],
                            xT_sb[kc][:, 128 + off:128 + off + 512],
                            start=(kc == 0), stop=(kc == 3))
                    nc.vector.tensor_scalar_add(
                        qT_sb[m][:, off:off + 512], ps[:], bq_sb[:, m:m + 1])
                for off, w in ((0, 512), (512, 512), (1024, 256)):
                    ps = pproj.tile([128, 512], F32, tag="ps")
                    for kc in range(4):
                        nc.tensor.matmul(
                            ps[:, :w], wk_sb[kc][:, 128 * m:128 * m + 128],
                            xT_sb[kc][:, off:off + w],
                            start=(kc == 0), stop=(kc == 3))
                    nc.vector.tensor_scalar_add(
                        kT_sb[m][:, off:off + w], ps[:, :w], bk_sb[:, m:m + 1])

            # ---------------- V projection (natural layout, hv-masked bias)
            v_sb = [bpool.tile([128, 8 * 66], F16, tag=f"v{j}", name=f"v{j}") for j in range(10)]
            for j in range(10):
                ps = pproj.tile([128, 512], F32, tag="ps")
                for kc in range(4):
                    nc.tensor.matmul(
                        ps[:], xT_sb[kc][:, 128 * j:128 * j + 128], wv_sb[kc][:],
                        start=(kc == 0), stop=False)
                nc.tensor.matmul(
                    ps[:], hv_sb[0:1, 128 * j:128 * j + 128], bv_sb[:],
                    start=False, stop=True)
                nc.vector.tensor_copy(
                    v_sb[j].rearrange("p (h c) -> p h c", c=66)[:, :, 0:64],
                    ps.rearrange("p (h c) -> p h c", c=64)[:, :, :])
                nc.vector.tensor_copy(
                    v_sb[j].rearrange("p (h c) -> p h c", c=66)[:, :, 64:65],
                    hvT8_sb[:, 8 * j:8 * j + 8].rearrange("p (a b) -> p a b", b=1))

            # ---------------- banded attention ---------------------------
            outT_sb = [bpool.tile([128, ROWS], F16, tag=f"oT{m}", name=f"oT{m}") for m in range(4)]
            if mode == "proj":
                for m in range(4):
                    nc.vector.tensor_copy(outT_sb[m][:], qT_sb[m][:])
            for b in range(NBLK if mode != "proj" else 0):
                for h in range(HEADS):
                    m, po = h // 2, 64 * (h % 2)
                    pe = pepool.tile([128, 384], F32, tag="pe")
                    for c in range(3):
                        nc.tensor.matmul(
                            pe[:, 128 * c:128 * c + 128],
                            kT_sb[m][po:po + 64, 128 * (b + c):128 * (b + c) + 128],
                            qT_sb[m][po:po + 64, 128 * b:128 * b + 128],
                            start=True, stop=True)
                    ex = wpool.tile([128, 384], F16, tag="ex")
                    nc.scalar.activation(
                        ex[:], pe[:], mybir.ActivationFunctionType.Exp, scale=SCALE)
                    nc.vector.tensor_mul(ex[:, 0:128], ex[:, 0:128], m0[:])
                    nc.vector.tensor_mul(ex[:, 256:384], ex[:, 256:384], m2[:])
                    pot = popool.tile([65, 128], F32, tag="po")
                    for c in range(3):
                        nc.tensor.matmul(
                            pot[:], v_sb[b + c][:, 66 * h:66 * h + 65],
                            ex[:, 128 * c:128 * c + 128],
                            start=(c == 0), stop=(c == 2))
                    if mode == "nonorm":
                        nc.vector.tensor_copy(
                            outT_sb[m][po:po + 64, 128 * b:128 * b + 128],
                            pot[0:64, :])
                    else:
                        # Newton-Raphson 1/d with native ALU ops only (the
                        # custom-DVE InstReciprocal ucode hangs on this path).
                        # Sign carried negated; the -1s rbp matmul flips it.
                        I32 = mybir.dt.int32
                        d_sb = wpool.tile([1, 128], F32, tag="dsb", name="dsb")
                        nc.vector.tensor_copy(d_sb[:], pot[64:65, :])
                        sd = wpool.tile([1, 128], F32, tag="sd", name="sd")
                        nc.vector.tensor_scalar(
                            sd[:].bitcast(I32), d_sb[:].bitcast(I32),
                            -1, None, mybir.AluOpType.bitwise_xor)
                        nc.vector.tensor_scalar(
                            sd[:].bitcast(I32), sd[:].bitcast(I32),
                            0x7EF311C4, None, mybir.AluOpType.add)
                        t1 = wpool.tile([1, 128], F32, tag="t1", name="t1")
                        nc.vector.tensor_mul(t1[:], d_sb[:], sd[:])
                        r1n = wpool.tile([1, 128], F32, tag="r1n", name="r1n")
                        nc.vector.scalar_tensor_tensor(
                            r1n[:], t1[:], 2.0, sd[:],
                            op0=mybir.AluOpType.subtract, op1=mybir.AluOpType.mult)
                        t3 = wpool.tile([1, 128], F32, tag="t3", name="t3")
                        nc.vector.tensor_mul(t3[:], d_sb[:], r1n[:])
                        r = wpool.tile([1, 128], F16, tag="r", name="r")
                        with nc.allow_low_precision(reason="fp16 softmax recip"):
                            nc.vector.scalar_tensor_tensor(
                                r[:], t3[:], 2.0, r1n[:],
                                op0=mybir.AluOpType.add, op1=mybir.AluOpType.mult)
                        rbp = popool.tile([64, 128], F32, tag="rb", name="rbp")
                        nc.tensor.matmul(rbp[:], negs_row[0:1, 0:64], r[:],
                                         start=True, stop=True)
                        rbs = wpool.tile([64, 128], F32, tag="rbs", name="rbs")
                        nc.vector.tensor_copy(rbs[:], rbp[:])
                        nc.vector.tensor_mul(
                            outT_sb[m][po:po + 64, 128 * b:128 * b + 128],
                            pot[0:64, :], rbs[:])

            # ---------------- output projection ---------------------------
            for nb in range(NBLK):
                ps = pproj.tile([128, 512], F32, tag="ps")
                for m in range(4):
                    nc.tensor.matmul(
                        ps[:], outT_sb[m][:, 128 * nb:128 * nb + 128], wo_sb[m][:],
                        start=(m == 0), stop=False)
                nc.tensor.matmul(ps[:], ones_row[:], bo_sb[:], start=False, stop=True)
                fin = fpool.tile([128, 512], F32, tag="fin")
                nc.vector.tensor_copy(fin[:], ps[:])
                nc.sync.dma_start(out[128 * nb:128 * nb + 128, :], fin[:])
    if not nc.is_finalized():
        nc.finalize()
    return nc


def make_in_maps(x, Wq, bq, Wk, bk, Wv, bv, Wo, bo):
    x = np.asarray(x, np.float32)
    shared = {
        "wqT": np.asarray(Wq, np.float32).T.astype(np.float16).copy(),
        "wkT": np.asarray(Wk, np.float32).T.astype(np.float16).copy(),
        "wvT": np.asarray(Wv, np.float32).T.astype(np.float16).copy(),
        "woT": np.asarray(Wo, np.float32).T.astype(np.float16).copy(),
        "bqc": np.asarray(bq, np.float32).reshape(4, 128).T.copy(),
        "bkc": np.asarray(bk, np.float32).reshape(4, 128).T.copy(),
        "bvr": np.asarray(bv, np.float16).reshape(1, EMB).copy(),
        "bor": np.asarray(bo, np.float16).reshape(1, EMB).copy(),
    }
    in_maps = []
    for core in range(NCORES):
        bi, sblk = core // 4, core % 4
        r0 = ROWS * sblk
        lo, hi = r0 - WINDOW, r0 + ROWS + WINDOW
        xh = np.zeros((HALO, EMB), np.float16)
        hv = np.zeros((HALO,), np.float16)
        clo, chi = max(lo, 0), min(hi, SEQ)
        xh[clo - lo:chi - lo] = x[bi, clo:chi].astype(np.float16)
        hv[clo - lo:chi - lo] = 1.0
        hvT8 = np.repeat(hv.reshape(10, 128).T[:, :, None], 8, axis=2)  # [128,10,8]
        in_maps.append(dict(
            shared,
            xT=xh.T.copy(),
            hvrow=hv.reshape(1, HALO).copy(),
            hvT8=hvT8.reshape(128, 80).copy(),
        ))
    return in_maps


def run(inputs, trace=False):
    nc = build_bass()
    in_maps = make_in_maps(**inputs)
    res = run_bass_kernel_spmd(nc, in_maps, list(range(NCORES)), trace=trace)
    out = np.zeros((BATCH, SEQ, EMB), np.float32)
    for core in range(NCORES):
        bi, sblk = core // 4, core % 4
        out[bi, ROWS * sblk:ROWS * (sblk + 1)] = res.results[core]["out"]
    return out, res


def kernel(**inputs):
    out, _ = run(inputs)
    return out
